# revision 16
# baseline (speedup 1.0000x reference)
"""Trainium2 Bass kernel for nn_CLFMv2_NoTemporalEmb (graph-PDE message passing).

Strategy: data-parallel over batch B=64 across 8 NeuronCores (8 batches/core).
Per core, activations are "pair-packed feature-major":
    tensor[psi, n],  psi = (batch_parity)*64 + d  (128 partitions),
    one [128, 1024] tensor per batch-pair (4 pairs/core).
Weight matmuls use block-diagonal [128,128] stationary operands so K=128,
M=128, PSUM dst partition 0. The Laplacian A@field uses PE-transposed field
tiles (regular matmuls against identity so HAM stays warm) as stationary
operands against the host-precomputed alpha*dt*(softmax(adj) - I) transpose;
the softmax and all weight packing run on host in float64.
Matmuls run in bf16 (full PE rate); PSUM accumulates fp32.
"""

import os
import contextlib

import numpy as np

import concourse.bacc as bacc
import concourse.tile as tile
import concourse.mybir as mybir
from concourse.bass_utils import run_bass_kernel_spmd

F32 = mybir.dt.float32
F32R = mybir.dt.float32r
BF16 = mybir.dt.bfloat16
MMDT = F32R if os.environ.get("KMM_DTYPE", "bf16") == "f32r" else BF16
AF = mybir.ActivationFunctionType
ALU = mybir.AluOpType

B, L, N, D, H, O = 64, 12, 1024, 64, 128, 12
STEPS = 4
NCORES = 8
BL = B // NCORES          # 8 batches per core
PAIRS = BL // 2           # 4
KCH = N // 128            # 8 adjacency chunks

# weight-pack slot order (each slot is a [128, 128] block in wpk)
WNAMES = ["w1eA", "w1eB", "w2eA", "w2eB", "pw1A", "pw1B", "pw2A", "pw2B",
          "wzbd", "uzbd", "whbd", "uhbd", "wobd", "dw1A", "dw1B",
          "dw2A", "dw2B", "ieye"]
BNAMES = ["eb1A", "eb1B", "eb2", "pb1A", "pb1B", "bz", "bh", "bo",
          "db1A", "db1B", "db2"]


def _build():
    nc = bacc.Bacc("TRN2", target_bir_lowering=False, debug=False)

    wpk = nc.dram_tensor("wpk", [128, len(WNAMES) * 128], MMDT,
                         kind="ExternalInput")
    bpk = nc.dram_tensor("bpk", [128, len(BNAMES)], F32, kind="ExternalInput")
    hist = nc.dram_tensor("hist", [BL, L, N], MMDT, kind="ExternalInput")
    ath = nc.dram_tensor("ath", [128, KCH * N], MMDT, kind="ExternalInput")
    out = nc.dram_tensor("out", [BL, O, N], F32, kind="ExternalOutput")

    with tile.TileContext(nc) as tc:
        with contextlib.ExitStack() as ctx:
            pp = ctx.enter_context(tc.tile_pool(name="persist", bufs=1))
            hab = ctx.enter_context(tc.tile_pool(name="hab", bufs=10))
            ftp = ctx.enter_context(tc.tile_pool(name="ftp", bufs=5))
            tmp = ctx.enter_context(tc.tile_pool(name="tmp", bufs=2))
            fep = ctx.enter_context(tc.tile_pool(name="fep", bufs=5))
            zcp = ctx.enter_context(tc.tile_pool(name="zcp", bufs=4))
            x2p = ctx.enter_context(tc.tile_pool(name="x2p", bufs=2))
            o2p = ctx.enter_context(tc.tile_pool(name="o2p", bufs=2))
            psA = ctx.enter_context(tc.tile_pool(name="psA", bufs=2, space="PSUM"))
            psB = ctx.enter_context(tc.tile_pool(name="psB", bufs=2, space="PSUM"))

            # ---- packed weights and biases: two DMAs ----
            wpkt = pp.tile([128, len(WNAMES) * 128], MMDT, tag="wpk", name="wpkt")
            nc.sync.dma_start(wpkt[:], wpk[:, :])
            bpkt = pp.tile([128, len(BNAMES)], F32, tag="bpk", name="bpkt")
            nc.sync.dma_start(bpkt[:], bpk[:, :])

            wt = {}
            for i, name in enumerate(WNAMES):
                if name in ("w1eA", "w1eB"):
                    wt[name] = wpkt[0:2 * L, i * 128:(i + 1) * 128]
                elif name in ("dw2A", "dw2B"):
                    wt[name] = wpkt[:, i * 128:i * 128 + 2 * O]
                else:
                    wt[name] = wpkt[:, i * 128:(i + 1) * 128]
            bs = {}
            for j, name in enumerate(BNAMES):
                if name == "db2":
                    bs[name] = bpkt[0:2 * O, j:j + 1]
                else:
                    bs[name] = bpkt[:, j:j + 1]

            # per-pair persistent activations
            field = [pp.tile([128, N], MMDT, tag=f"field{p}", name=f"field{p}")
                     for p in range(PAIRS)]
            state = [pp.tile([128, N], MMDT, tag=f"state{p}", name=f"state{p}")
                     for p in range(PAIRS)]

            # ---- encoder (emitted before the big AT DMA) ----
            for p in range(PAIRS):
                xp = x2p.tile([2 * L, N], MMDT, tag="x2p", name="xp")
                nc.sync.dma_start(xp[0:L, :], hist[2 * p, :, :])
                nc.sync.dma_start(xp[L:2 * L, :], hist[2 * p + 1, :, :])
                hea = hab.tile([128, N], MMDT, tag="hab", name="hea")
                heb = hab.tile([128, N], MMDT, tag="hab", name="heb")
                for (wname, bname, dst) in [("w1eA", "eb1A", hea),
                                            ("w1eB", "eb1B", heb)]:
                    ph = psA.tile([128, N], F32, tag="psA", name="psah")
                    for hf in range(2):
                        sl = slice(hf * 512, (hf + 1) * 512)
                        nc.tensor.matmul(ph[:, sl], wt[wname], xp[:, sl],
                                         start=True, stop=True)
                    nc.scalar.activation(dst[:], ph[:], AF.Relu, bias=bs[bname])
                pf = psB.tile([128, N], F32, tag="psB", name="psbf")
                for hf in range(2):
                    sl = slice(hf * 512, (hf + 1) * 512)
                    nc.tensor.matmul(pf[:, sl], wt["w2eA"], hea[:, sl],
                                     start=True, stop=False)
                    nc.tensor.matmul(pf[:, sl], wt["w2eB"], heb[:, sl],
                                     start=False, stop=True)
                nc.scalar.activation(field[p][:], pf[:], AF.Identity,
                                     bias=bs["eb2"])

            # ---- adjacency operator: host-precomputed, one DMA ----
            AT = pp.tile([128, KCH * N], MMDT, tag="AT", name="AT")
            nc.sync.dma_start(AT[:], ath[:, :])

            def emit_transpose(p):
                ptr = psA.tile([128, N], F32, tag="psA", name="psatr")
                for k in range(KCH):
                    nc.tensor.matmul(ptr[:, k * 128:(k + 1) * 128],
                                     field[p][:, k * 128:(k + 1) * 128],
                                     wt["ieye"], start=True, stop=True)
                ft = ftp.tile([128, N], MMDT, tag="ft", name="ft")
                nc.vector.tensor_copy(ft[:, 0:512], ptr[:, 0:512])
                nc.scalar.activation(ft[:, 512:N], ptr[:, 512:N], AF.Copy)
                return ft

            ftq = [emit_transpose(p) for p in range(PAIRS)]

            # ---- main steps (phase-major software pipelining) ----
            for s in range(STEPS):
                first = (s == 0)
                fts, ftq = ftq, []
                has, hbs = [], []
                for p in range(PAIRS):
                    # B) pde layer 1: hA/hB = tanh(field @ w1 + b1)
                    ha = hab.tile([128, N], MMDT, tag="hab", name="ha")
                    hb = hab.tile([128, N], MMDT, tag="hab", name="hb")
                    for (wname, bname, dst) in [("pw1A", "pb1A", ha),
                                                ("pw1B", "pb1B", hb)]:
                        ph = psA.tile([128, N], F32, tag="psA", name="psah")
                        for hf in range(2):
                            sl = slice(hf * 512, (hf + 1) * 512)
                            nc.tensor.matmul(ph[:, sl], wt[wname],
                                             field[p][:, sl],
                                             start=True, stop=True)
                        nc.scalar.activation(dst[:], ph[:], AF.Tanh,
                                             bias=bs[bname])
                    has.append(ha)
                    hbs.append(hb)

                fes = []

                def emit_gru(p, fe_t):
                    z_t = zcp.tile([128, N], MMDT, tag="zc", name="z_t")
                    c_t = zcp.tile([128, N], MMDT, tag="zc", name="c_t")
                    for (wname, uname, bname, func, dst) in [
                        ("wzbd", "uzbd", "bz", AF.Sigmoid, z_t),
                        ("whbd", "uhbd", "bh", AF.Tanh, c_t),
                    ]:
                        pz = psB.tile([128, N], F32, tag="psB", name="psbz")
                        for hf in range(2):
                            sl = slice(hf * 512, (hf + 1) * 512)
                            nc.tensor.matmul(pz[:, sl], wt[wname], fe_t[:, sl],
                                             start=True, stop=first)
                            if not first:
                                nc.tensor.matmul(pz[:, sl], wt[uname],
                                                 state[p][:, sl],
                                                 start=False, stop=True)
                        nc.scalar.activation(dst[:], pz[:], func, bias=bs[bname])
                    if first:
                        nc.vector.tensor_tensor(state[p][:], z_t[:], c_t[:],
                                                ALU.mult)
                    else:
                        t1 = tmp.tile([128, N], MMDT, tag="tmp", name="t1")
                        nc.vector.tensor_tensor(t1[:], c_t[:], state[p][:],
                                                ALU.subtract)
                        nc.vector.tensor_tensor(t1[:], z_t[:], t1[:], ALU.mult)
                        nc.vector.tensor_tensor(state[p][:], state[p][:], t1[:],
                                                ALU.add)

                for p in range(PAIRS):
                    ft, ha, hb = fts[p], has[p], hbs[p]
                    # C) fe psum: Laplacian(-c_lap diag folded) + pde layer 2
                    fe_t = fep.tile([128, N], MMDT, tag="fe", name="fe_t")
                    pfe = psB.tile([128, N], F32, tag="psB", name="psbfe")
                    for hf in range(2):
                        sl = slice(hf * 512, (hf + 1) * 512)
                        for k in range(KCH):
                            nc.tensor.matmul(
                                pfe[:, sl],
                                ft[:, k * 128:(k + 1) * 128],
                                AT[:, k * N + hf * 512:k * N + (hf + 1) * 512],
                                start=(k == 0), stop=False)
                        nc.tensor.matmul(pfe[:, sl], wt["pw2A"], ha[:, sl],
                                         start=False, stop=False)
                        nc.tensor.matmul(pfe[:, sl], wt["pw2B"], hb[:, sl],
                                         start=False, stop=True)
                    # fe = psum + field (pb2 bias folded into bz/bh/bo)
                    nc.vector.tensor_tensor(fe_t[:], pfe[:], field[p][:],
                                            ALU.add)
                    fes.append(fe_t)
                    # D+E) GRU trails the Laplacian by two pairs for slack
                    if p >= 2:
                        emit_gru(p - 2, fes[p - 2])
                emit_gru(PAIRS - 2, fes[PAIRS - 2])
                emit_gru(PAIRS - 1, fes[PAIRS - 1])

                def emit_dec(p):
                    dha = hab.tile([128, N], MMDT, tag="hab", name="dha")
                    dhb = hab.tile([128, N], MMDT, tag="hab", name="dhb")
                    for (wname, bname, dst) in [("dw1A", "db1A", dha),
                                                ("dw1B", "db1B", dhb)]:
                        ph = psA.tile([128, N], F32, tag="psA", name="psah")
                        for hf in range(2):
                            sl = slice(hf * 512, (hf + 1) * 512)
                            nc.tensor.matmul(ph[:, sl], wt[wname],
                                             field[p][:, sl],
                                             start=True, stop=True)
                        nc.scalar.activation(dst[:], ph[:], AF.Relu,
                                             bias=bs[bname])
                    po = psB.tile([2 * O, N], F32, tag="psB", name="psbo")
                    for hf in range(2):
                        sl = slice(hf * 512, (hf + 1) * 512)
                        nc.tensor.matmul(po[:, sl], wt["dw2A"], dha[:, sl],
                                         start=True, stop=False)
                        nc.tensor.matmul(po[:, sl], wt["dw2B"], dhb[:, sl],
                                         start=False, stop=True)
                    o2 = o2p.tile([2 * O, N], F32, tag="o2", name="o2")
                    nc.scalar.activation(o2[:], po[:], AF.Identity,
                                         bias=bs["db2"])
                    nc.sync.dma_start(out[2 * p, :, :], o2[0:O, :])
                    nc.sync.dma_start(out[2 * p + 1, :, :], o2[O:2 * O, :])

                for p in range(PAIRS):
                    # F) field' = fe + state @ wo + bo
                    pf = psB.tile([128, N], F32, tag="psB", name="psbf2")
                    for hf in range(2):
                        sl = slice(hf * 512, (hf + 1) * 512)
                        nc.tensor.matmul(pf[:, sl], wt["wobd"], state[p][:, sl],
                                         start=True, stop=False)
                        nc.tensor.matmul(pf[:, sl], wt["ieye"], fes[p][:, sl],
                                         start=False, stop=True)
                    nc.scalar.activation(field[p][:], pf[:], AF.Identity,
                                         bias=bs["bo"])
                    if s < STEPS - 1:
                        ftq.append(emit_transpose(p))
                    else:
                        emit_dec(p)

            # ---- decoder (now emitted inside the last step) ----
            for p in []:
                dha = hab.tile([128, N], MMDT, tag="hab", name="dha")
                dhb = hab.tile([128, N], MMDT, tag="hab", name="dhb")
                for (wname, bname, dst) in [("dw1A", "db1A", dha),
                                            ("dw1B", "db1B", dhb)]:
                    ph = psA.tile([128, N], F32, tag="psA", name="psah")
                    for hf in range(2):
                        sl = slice(hf * 512, (hf + 1) * 512)
                        nc.tensor.matmul(ph[:, sl], wt[wname], field[p][:, sl],
                                         start=True, stop=True)
                    nc.scalar.activation(dst[:], ph[:], AF.Relu, bias=bs[bname])
                po = psB.tile([2 * O, N], F32, tag="psB", name="psbo")
                for hf in range(2):
                    sl = slice(hf * 512, (hf + 1) * 512)
                    nc.tensor.matmul(po[:, sl], wt["dw2A"], dha[:, sl],
                                     start=True, stop=False)
                    nc.tensor.matmul(po[:, sl], wt["dw2B"], dhb[:, sl],
                                     start=False, stop=True)
                o2 = o2p.tile([2 * O, N], F32, tag="o2", name="o2")
                nc.scalar.activation(o2[:], po[:], AF.Identity, bias=bs["db2"])
                nc.sync.dma_start(out[2 * p, :, :], o2[0:O, :])
                nc.sync.dma_start(out[2 * p + 1, :, :], o2[O:2 * O, :])

    nc.compile()
    return nc


MMNP = mybir.dt.np(MMDT)


def _blockdiag(w):
    w = np.asarray(w, dtype=np.float64)
    r, c = w.shape
    o = np.zeros((2 * r, 2 * c), dtype=np.float64)
    o[:r, :c] = w
    o[r:, c:] = w
    return o


def _slot(w):
    """place an array into a [128, 128] weight slot."""
    w = np.asarray(w, dtype=np.float64)
    o = np.zeros((128, 128), dtype=np.float64)
    o[:w.shape[0], :w.shape[1]] = w
    return o


def prepare(inputs):
    """Host packing (float64) + compiled Bass module + per-core input maps."""
    g = {k: np.asarray(v) for k, v in inputs.items()}
    pde_mix = float(np.asarray(g["pde_mix"], dtype=np.float64))
    alpha = float(1.0 / (1.0 + np.exp(-pde_mix)))
    dt_ = 1.0 / STEPS
    s2 = (1.0 - alpha) * dt_
    c_lap = alpha * dt_

    f64 = lambda k: np.asarray(g[k], np.float64)
    enc_w1, enc_w2 = f64("enc_w1"), f64("enc_w2")
    pde_w1, pde_w2 = f64("pde_w1"), f64("pde_w2") * s2
    dec_w1, dec_w2 = f64("dec_w1"), f64("dec_w2")

    slots = {
        "w1eA": _blockdiag(enc_w1[:, 0:64]),
        "w1eB": _blockdiag(enc_w1[:, 64:128]),
        "w2eA": _blockdiag(enc_w2[0:64, :]),
        "w2eB": _blockdiag(enc_w2[64:128, :]),
        "pw1A": _blockdiag(pde_w1[:, 0:64]),
        "pw1B": _blockdiag(pde_w1[:, 64:128]),
        "pw2A": _blockdiag(pde_w2[0:64, :]),
        "pw2B": _blockdiag(pde_w2[64:128, :]),
        "wzbd": _blockdiag(f64("ss_wz")),
        "uzbd": _blockdiag(f64("ss_uz")),
        "whbd": _blockdiag(f64("ss_wh")),
        "uhbd": _blockdiag(f64("ss_uh")),
        "wobd": _blockdiag(f64("ss_wo")),
        "dw1A": _blockdiag(dec_w1[:, 0:64]),
        "dw1B": _blockdiag(dec_w1[:, 64:128]),
        "dw2A": _blockdiag(dec_w2[0:64, :]),
        "dw2B": _blockdiag(dec_w2[64:128, :]),
        "ieye": np.eye(128, dtype=np.float64),
    }
    wpk = np.concatenate([_slot(slots[n]) for n in WNAMES], axis=1)

    # biases; pb2 folded into bz/bh/bo (fe carries no bias on device)
    pb2d = f64("pde_b2") * s2
    bz_f = f64("ss_bz") + pb2d @ f64("ss_wz")
    bh_f = f64("ss_bh") + pb2d @ f64("ss_wh")
    bo_f = f64("ss_bo") + pb2d
    bias_vals = {
        "eb1A": np.tile(f64("enc_b1")[0:64], 2),
        "eb1B": np.tile(f64("enc_b1")[64:128], 2),
        "eb2": np.tile(f64("enc_b2"), 2),
        "pb1A": np.tile(f64("pde_b1")[0:64], 2),
        "pb1B": np.tile(f64("pde_b1")[64:128], 2),
        "bz": np.tile(bz_f, 2),
        "bh": np.tile(bh_f, 2),
        "bo": np.tile(bo_f, 2),
        "db1A": np.tile(f64("dec_b1")[0:64], 2),
        "db1B": np.tile(f64("dec_b1")[64:128], 2),
        "db2": np.tile(f64("dec_b2"), 2),
    }
    bpk = np.zeros((128, len(BNAMES)), dtype=np.float64)
    for j, name in enumerate(BNAMES):
        v = bias_vals[name]
        bpk[:len(v), j] = v

    # adjacency operator: softmax rows, scale, subtract diag, transpose
    adj64 = f64("adj")
    e = np.exp(adj64 - adj64.max(axis=-1, keepdims=True))
    A = e / e.sum(axis=-1, keepdims=True)
    M = c_lap * (A - np.eye(N))
    ath = M.T.reshape(KCH, 128, N).transpose(1, 0, 2).reshape(128, KCH * N)

    common = {
        "wpk": np.ascontiguousarray(wpk.astype(np.float32)).astype(MMNP),
        "bpk": np.ascontiguousarray(bpk.astype(np.float32)),
        "ath": np.ascontiguousarray(ath.astype(np.float32)).astype(MMNP),
    }

    hist = np.asarray(g["history_data"], np.float32)[..., 0]  # [B, L, N]
    in_maps = []
    for c in range(NCORES):
        m = dict(common)
        m["hist"] = np.ascontiguousarray(hist[c * BL:(c + 1) * BL]).astype(MMNP)
        in_maps.append(m)

    nc = _build()
    return nc, in_maps


def assemble(results):
    outs = [results[c]["out"] for c in range(NCORES)]          # [BL, O, N]
    full = np.concatenate(outs, axis=0)                        # [B, O, N]
    return np.ascontiguousarray(full[..., None].astype(np.float32))


def kernel(**inputs) -> np.ndarray:
    nc, in_maps = prepare(inputs)
    res = run_bass_kernel_spmd(nc, in_maps, core_ids=list(range(NCORES)))
    return assemble(res.results)
